# revision 4
# baseline (speedup 1.0000x reference)
"""KAN layer (B-spline + silu) Trainium2 kernel, 8-way tensor-parallel.

Math (uniform knot grid):
  Truncated-power features S_i(v) = relu(v - i)^3, v = (x - t0)/h, i = 0..14,
  give the cubic B-spline basis via the banded map  B_f = sum_r w5[r] S_{f+r}
  (w5 = [1,-4,6,-4,1]/6).  That banded combine is FOLDED INTO THE WEIGHTS on
  the host:  out[n, j*256+q] = sum_p S_p(v[n,j]) * Cw'[p, j*256+q]
                               + silu(x[n,j]) * W[j*256+q],
  with  Cw' = M @ (C * W)  (M the 15x11 w5 band matrix) computed in f64.
  The device then runs a single fp16 matmul per output tile: the S features
  are computed as relu -> square -> mul (f32 chain, fp16 result).  fp16 S is
  accurate enough because the spline term is only ~0.6% of the output norm
  (xavier init over the 65536-wide fan-out makes C*W tiny vs W*silu).
  fp16 scaling: weights stored as 32*Cw' / 32*W (out of the fp16 subnormal
  range); the S chain carries 1/32 via a cbrt(32) fold into the relu
  scale/bias, and silu is pre-scaled by 1/32.  PSUM f32 = unscaled output,
  cast to fp16 on evacuation and written to HBM in fp16 (halving the HBM
  write floor, which dominates), widened to f32 on the host.

Sharding: core s owns j in [32s, 32s+32) (columns [8192s, 8192(s+1)) of the
flattened output).  Per core, j's are grouped into 4 octets of 8; within an
octet, j-pairs map to the 4 PE row groups.  Row layout per 32-row group:
  S tile (fp16): [15 S(j_a), 15 S(j_b), silu'(j_a), silu'(j_b)]
The main matmul (K=32, tile_position (32r,0), 4 row bands concurrent on the
PE) streams 512 cols per group; rhs rows 30/31 hold 32*W.

Performance structure (per core):
  - n is processed in 8 chunks of 256 rows; partition p of chunk c holds
    output rows 256c + 2p + t (t = 0..1) — matmuls take stride-2 column
    slices of the S tile.  Stores are [128, 2048] half-chunks (4 KB per
    partition) issued right after the two evac copies of that t-half,
    alternating between the sync (t=0) and gpsimd (t=1) DMA queues.
  - PSUM is one pool of four 2-bank tiles, all available to the main
    matmul (no combine stage).  Evacuation (f32 -> fp16 cast) alternates
    scalar/vector.
  - Input loads are spread across four queues at t=0 (bias on scalar, silu
    on vector, weights on sync, x-replicas on gpsimd) so everything lands
    during the compute fill.  silu rows are scattered SBUF->SBUF on the
    sync queue.
  - Octet 0's chain runs half-tiles (relu/square/mul + scatter per half) so
    the first matmuls start ~4 us in; octet o+1's chain spreads across
    octet o's chunk stream.
"""

import numpy as np

import concourse.bass as bass
import concourse.bacc as bacc
import concourse.tile as tile
from concourse import mybir
from concourse.bass_utils import run_bass_kernel_spmd

N = 2048          # batch
N_IN = 256
N_OUT = 256
NCORES = 8
JPC = N_IN // NCORES      # 32 j per core
NOCT = JPC // 8           # 4 octets of 8 j's
NCH = N // 256            # 8 n-chunks of 256 rows
F32 = mybir.dt.float32
F16 = mybir.dt.float16
WSCALE = 32.0             # fp16 weight scale (S/silu carry 1/32)


def _build_bass(scale_val: float):
    nc = bacc.Bacc(trn_type="TRN2")

    xrep = nc.dram_tensor("xrep", [NOCT, 128, N], F16, kind="ExternalInput")
    biasv = nc.dram_tensor("biasv", [128, 1], F32, kind="ExternalInput")
    rhsp = nc.dram_tensor("rhsp", [128, NOCT * 512], F16, kind="ExternalInput")
    siluT = nc.dram_tensor("siluT", [JPC, N], F16, kind="ExternalInput")
    # out[o, c, t, p, col] = row n = 256c + 2p + t, col 2048o + col
    out = nc.dram_tensor("out", [NOCT, NCH, 2, 128, 2048], F16,
                         kind="ExternalOutput")

    with tile.TileContext(nc) as tc:
        with (
            tc.tile_pool(name="consts", bufs=1) as consts,
            tc.tile_pool(name="xin", bufs=4) as xin,
            tc.tile_pool(name="chain", bufs=2) as chain,
            tc.tile_pool(name="ss", bufs=1) as sspool,
            tc.tile_pool(name="stage", bufs=8) as stage_pool,
            tc.tile_pool(name="psum", bufs=4, space="PSUM") as psum_pool,
        ):
            # Input loads fan out across queues: scalar=bias, vector=silu,
            # sync=weights, gpsimd=x replicas.  All land during the fill.
            bias_sb = consts.tile([128, 1], F32, name="bias_sb")
            nc.scalar.dma_start(out=bias_sb, in_=biasv[:, :])
            silu_sb = consts.tile([JPC, N], F16, name="silu_sb")
            nc.scalar.dma_start(out=silu_sb, in_=siluT[:, :])
            rhs_sb = consts.tile([128, NOCT * 512], F16, name="rhs_sb")
            nc.sync.dma_start(out=rhs_sb, in_=rhsp[:, :])
            xr_tiles = []
            for o in range(NOCT):
                xr = xin.tile([128, N], F16, tag=f"xr{o}", name=f"xr{o}")
                xr_tiles.append(xr)
                nc.gpsimd.dma_start(out=xr, in_=xrep[o])

            ss_tiles = [None] * NOCT
            chain_t = [None] * NOCT
            cnt = 0

            def emit_chain_step(o, step):
                # 6 half-tile steps ([128, 1024] each):
                #   0/1: relu halves (scalar), 2/3: square halves (scalar),
                #   4/5: mul halves (vector, fp16 out into the S tile).
                h = N // 2
                lo, hi = (0, h) if step % 2 == 0 else (h, N)
                if step < 2:
                    if chain_t[o] is None:
                        t1 = chain.tile([128, N], F32, tag="t1", name=f"t1_{o}")
                        chain_t[o] = [t1, None]
                    t1 = chain_t[o][0]
                    nc.scalar.activation(
                        t1[:, lo:hi], xr_tiles[o][:, lo:hi],
                        mybir.ActivationFunctionType.Relu,
                        bias=bias_sb[:, 0:1], scale=scale_val,
                    )
                elif step < 4:
                    if chain_t[o][1] is None:
                        chain_t[o][1] = chain.tile([128, N], F32, tag="t2",
                                                   name=f"t2_{o}")
                    t1, t2 = chain_t[o]
                    nc.scalar.square(t2[:, lo:hi], t1[:, lo:hi])
                else:
                    if ss_tiles[o] is None:
                        ss_tiles[o] = sspool.tile([128, N], F16, tag=f"ss{o}",
                                                  name=f"ss{o}")
                    t1, t2 = chain_t[o]
                    nc.vector.tensor_mul(ss_tiles[o][:, lo:hi], t1[:, lo:hi],
                                         t2[:, lo:hi])

            def emit_scatter(o, lo, hi):
                # silu rows 30/31 of each 32-row group, SBUF->SBUF on sync.
                ss = ss_tiles[o]
                for r in range(4):
                    nc.sync.dma_start(
                        out=ss[32 * r + 30 : 32 * r + 32, lo:hi],
                        in_=silu_sb[8 * o + 2 * r : 8 * o + 2 * r + 2, lo:hi],
                    )

            def emit_main_chunk(o, c):
                nonlocal cnt
                ss = ss_tiles[o]
                st = stage_pool.tile([128, 4096], F16, tag="st",
                                     name=f"st{o}_{c}")
                for t in range(2):       # row residue: n = 256c + 2p + t
                    for rp in range(2):  # row-group pairs (2rp, 2rp+1)
                        ps = psum_pool.tile([128, 1024], F32, tag="ps",
                                            name=f"ps{o}_{c}_{t}_{rp}")
                        for rr in range(2):
                            r = 2 * rp + rr
                            nc.tensor.matmul(
                                ps[:, 512 * rr : 512 * (rr + 1)],
                                lhsT=ss[32 * r : 32 * r + 32,
                                        256 * c + t : 256 * (c + 1) : 2],
                                rhs=rhs_sb[32 * r : 32 * r + 32,
                                           512 * o : 512 * (o + 1)],
                                start=True,
                                stop=True,
                                tile_position=(32 * r, 0),
                            )
                        dst = st[:, 2048 * t + 1024 * rp :
                                 2048 * t + 1024 * (rp + 1)]
                        if cnt % 2 == 0:
                            nc.vector.tensor_copy(dst, ps)
                        else:
                            nc.scalar.copy(dst, ps)
                        cnt += 1
                    # Half-chunk store right after its two evacs; the two
                    # halves ride different descriptor-gen queues.
                    deng = nc.sync if t == 0 else nc.gpsimd
                    deng.dma_start(out=out[o, c, t],
                                   in_=st[:, 2048 * t : 2048 * (t + 1)])

            # Octet 0 fill: per-half chain + scatter so chunks 0-3 start
            # as soon as the first half of the S tile is ready.
            emit_chain_step(0, 0)
            emit_chain_step(0, 2)
            emit_chain_step(0, 4)
            emit_scatter(0, 0, N // 2)
            emit_chain_step(0, 1)
            emit_chain_step(0, 3)
            emit_chain_step(0, 5)
            emit_scatter(0, N // 2, N)

            # Wavefront: octet o's 8 chunks carry octet o+1's chain steps.
            for o in range(NOCT):
                for c in range(NCH):
                    emit_main_chunk(o, c)
                    if o + 1 < NOCT:
                        s = c - 1
                        if 0 <= s < 6:
                            emit_chain_step(o + 1, s)
                        if c == NCH - 1:
                            emit_scatter(o + 1, 0, N)

    nc.compile()
    return nc


def _host_prep(x, C, W, grid):
    """Build per-core input maps."""
    t0 = np.float64(grid[0, 0])
    h = np.float64(grid[0, 1] - grid[0, 0])
    cbrt = np.float64(WSCALE) ** (1.0 / 3.0)
    w5 = np.array([1.0, -4.0, 6.0, -4.0, 1.0], np.float64) / 6.0

    # Fold the banded combine into the weights (f64):
    #   Cw'[p, col] = sum_f M[p, f] * (C*W)[f, col],  M[f+r, f] = w5[r].
    M = np.zeros((15, 11), np.float64)
    for f in range(11):
        for r in range(5):
            M[f + r, f] = w5[r]
    CW = C.astype(np.float64) * W.astype(np.float64)        # (11, 65536)
    Cwp32 = (M @ CW * WSCALE).astype(np.float16)            # (15, 65536)
    W32 = (W.astype(np.float64) * WSCALE).astype(np.float16)

    xd = x.astype(np.float64)
    silu_p = (xd / (1.0 + np.exp(-xd)) / WSCALE).astype(np.float16)  # silu/32

    # S-tile partition layout within a 32-row group:
    #   s in [0,15)  -> S_i of j_a (i = s)
    #   s in [15,30) -> S_i of j_b (i = s - 15)
    #   s = 30/31    -> silu'(j_a)/silu'(j_b) (scatter; relu bias -64 ->
    #                   the chain writes exact zeros there first)
    s_idx = np.arange(128) % 32
    feat_i = np.where(s_idx < 15, s_idx, np.where(s_idx < 30, s_idx - 15, 0))
    which_b = np.where(s_idx < 15, 0, np.where(s_idx < 30, 1, s_idx - 30))
    biasv = np.where(
        s_idx < 30, (-t0 / h - feat_i) / cbrt, -64.0
    ).astype(np.float32).reshape(128, 1)
    scale_val = float(np.float32(1.0 / h / cbrt))

    x16 = x.astype(np.float16)
    in_maps = []
    for s in range(NCORES):
        jb = JPC * s
        xt = np.ascontiguousarray(x16[:, jb : jb + JPC].T)    # (32, N) fp16
        xrep = np.empty((NOCT, 128, N), np.float16)
        rgrp = np.arange(128) // 32
        for o in range(NOCT):
            jloc = 8 * o + 2 * rgrp + which_b
            xrep[o] = xt[jloc]
        silu_t = np.ascontiguousarray(silu_p[:, jb : jb + JPC].T)  # (32, N)

        # rhs row layout per group: [15 Cw'a, 15 Cw'b, W a, W b] (x32)
        rhsp = np.zeros((128, NOCT * 512), np.float16)
        for o in range(NOCT):
            for rr in range(4):
                ja = (jb + 8 * o + 2 * rr) * N_OUT
                jbc = (jb + 8 * o + 2 * rr + 1) * N_OUT
                base = 32 * rr
                rhsp[base : base + 15, 512 * o : 512 * o + 256] = \
                    Cwp32[:, ja : ja + 256]
                rhsp[base + 15 : base + 30, 512 * o + 256 : 512 * o + 512] = \
                    Cwp32[:, jbc : jbc + 256]
                rhsp[base + 30, 512 * o : 512 * o + 256] = W32[0, ja : ja + 256]
                rhsp[base + 31, 512 * o + 256 : 512 * o + 512] = \
                    W32[0, jbc : jbc + 256]
        in_maps.append({
            "xrep": np.ascontiguousarray(xrep),
            "biasv": biasv,
            "rhsp": np.ascontiguousarray(rhsp),
            "siluT": silu_t,
        })
    return in_maps, scale_val


def _assemble(out_core):
    """[NOCT, NCH, 2, 128, 2048] fp16 -> [N, 8192] (n = 256c + 2p + t)."""
    a = out_core.reshape(NOCT, NCH, 2, 128, 2048)
    return a.transpose(1, 3, 2, 0, 4).reshape(N, JPC * N_OUT)


def kernel(x, C, W, grid):
    in_maps, scale_val = _host_prep(
        np.asarray(x, np.float32), np.asarray(C, np.float32),
        np.asarray(W, np.float32), np.asarray(grid, np.float32),
    )
    nc = _build_bass(scale_val)
    res = run_bass_kernel_spmd(nc, in_maps, core_ids=list(range(NCORES)))
    return np.ascontiguousarray(np.concatenate(
        [_assemble(r["out"]).astype(np.float32) for r in res.results], axis=1))


if __name__ == "__main__":
    rng = np.random.default_rng(0)
    x = rng.standard_normal((N, N_IN), dtype=np.float32)
    C = rng.standard_normal((11, N_IN * N_OUT), dtype=np.float32) * 0.005
    W = rng.standard_normal((1, N_IN * N_OUT), dtype=np.float32) * 0.005
    knots = -5.25 + 0.75 * np.arange(15, dtype=np.float32)
    grid = np.tile(knots, (N_IN, 1))
    out = kernel(x, C, W, grid)
    print("kernel out:", out.shape, out.dtype, float(np.abs(out).mean()))


# revision 6
# speedup vs baseline: 1.0393x; 1.0393x over previous
"""KAN layer (B-spline + silu) Trainium2 kernel, 8-way tensor-parallel.

Math (uniform knot grid):
  Truncated-power features S_i(v) = relu(v - i)^3, v = (x - t0)/h, i = 0..14,
  give the cubic B-spline basis via the banded map  B_f = sum_r w5[r] S_{f+r}
  (w5 = [1,-4,6,-4,1]/6).  That banded combine is FOLDED INTO THE WEIGHTS on
  the host:  out[n, j*256+q] = sum_p S_p(v[n,j]) * Cw'[p, j*256+q]
                               + silu(x[n,j]) * W[j*256+q],
  with  Cw' = M @ (C * W)  (M the 15x11 w5 band matrix) computed in f64.
  The device runs a single fp16 matmul per output tile: the S features are
  computed as relu (scalar) -> square (gpsimd) -> mul (vector, fp16 out).
  fp16 S is accurate enough because the spline term is only ~0.6% of the
  output norm (xavier init over the 65536-wide fan-out makes C*W tiny).
  fp16 scaling: weights stored as 32*Cw' / 32*W (out of the fp16 subnormal
  range); the S chain carries 1/32 via a cbrt(32) fold into the relu
  scale/bias, and silu is pre-scaled by 1/32.  PSUM f32 = unscaled output,
  cast to fp16 on evacuation and written to HBM in fp16 (halving the HBM
  write floor, which dominates), widened to f32 on the host.

Sharding: core s owns j in [32s, 32s+32) (columns [8192s, 8192(s+1)) of the
flattened output).  Per core, j's are grouped into 4 octets of 8; within an
octet, j-pairs map to the 4 PE row groups.  Row layout per 32-row group:
  S tile (fp16): [15 S(j_a), 15 S(j_b), silu'(j_a), silu'(j_b)]

Performance structure (per core), tuned for the ~93 us HBM store floor:
  - n is processed in 8 chunks of 256 rows; partition p of chunk c holds
    output rows 256c + 2p + t (t = 0..1).  Per chunk and t-half, the four
    row-group matmuls (tile_position (32r,0), 4 bands concurrent) fill one
    [128, 2048] PSUM tile (4 banks; 2 tiles double-buffer), evacuated by a
    single f32->fp16 copy alternating scalar/vector 17:15 (vector is ~12%
    slower per element), then stored as a [128, 2048] half-chunk (4 KB per
    partition) on the sync (t=0) / gpsimd (t=1) queues.  The last chunk
    stores quarters on both queues to shorten the drain.
  - The square runs on gpsimd (SBUF-only engine, otherwise idle) to keep
    scalar/vector free for evacuation; gpsimd cannot read PSUM, so it
    cannot help with evacuation itself.
  - Octet 0's chain runs quarter-tiles (relu -> square -> mul -> silu
    scatter per 512 cols, each quarter feeding 2 chunks) to shorten the
    fill; octet o+1's chain (half-tiles) spreads across octet o's chunks.
  - Input DMAs: xr0 quarters then weights then xr1-3 on the sync queue,
    bias + silu table on the scalar queue - everything lands during the
    compute fill, so stores see no steady-state read interference.
"""

import numpy as np

import concourse.bass as bass
import concourse.bacc as bacc
import concourse.tile as tile
from concourse import mybir
from concourse.bass_utils import run_bass_kernel_spmd

N = 2048          # batch
N_IN = 256
N_OUT = 256
NCORES = 8
JPC = N_IN // NCORES      # 32 j per core
NOCT = JPC // 8           # 4 octets of 8 j's
NCH = N // 256            # 8 n-chunks of 256 rows
F32 = mybir.dt.float32
F16 = mybir.dt.float16
WSCALE = 32.0             # fp16 weight scale (S/silu carry 1/32)

# Evac engine pattern: scalar(Act) is faster per copy than vector(DVE);
# 17:15 over 32 half-chunks balances scalar = relu+evac vs vector = mul+evac.
EVAC_PERIOD = 32
EVAC_ACT = {i for i in range(0, EVAC_PERIOD, 2)} | {EVAC_PERIOD - 1}


def _build_bass(scale_val: float):
    nc = bacc.Bacc(trn_type="TRN2")

    xrep = nc.dram_tensor("xrep", [NOCT, 128, N], F16, kind="ExternalInput")
    biasv = nc.dram_tensor("biasv", [128, 1], F32, kind="ExternalInput")
    rhsp = nc.dram_tensor("rhsp", [128, NOCT * 512], F16, kind="ExternalInput")
    siluT = nc.dram_tensor("siluT", [JPC, N], F16, kind="ExternalInput")
    # out[o, c, t, p, col] = row n = 256c + 2p + t, col 2048o + col
    out = nc.dram_tensor("out", [NOCT, NCH, 2, 128, 2048], F16,
                         kind="ExternalOutput")

    with tile.TileContext(nc) as tc:
        with (
            tc.tile_pool(name="consts", bufs=1) as consts,
            tc.tile_pool(name="xin", bufs=4) as xin,
            tc.tile_pool(name="chain", bufs=2) as chain,
            tc.tile_pool(name="ss", bufs=1) as sspool,
            tc.tile_pool(name="stage", bufs=8) as stage_pool,
            tc.tile_pool(name="psum", bufs=2, space="PSUM") as psum_pool,
        ):
            # Fill-critical loads first: xr0 in quarters (sync), bias+silu
            # (scalar).  rhs and xr1-3 follow on sync/gpsimd.
            xr_tiles = [xin.tile([128, N], F16, tag=f"xr{o}", name=f"xr{o}")
                        for o in range(NOCT)]
            for q in range(4):
                nc.sync.dma_start(out=xr_tiles[0][:, 512 * q : 512 * (q + 1)],
                                  in_=xrep[0, :, 512 * q : 512 * (q + 1)])
            bias_sb = consts.tile([128, 1], F32, name="bias_sb")
            nc.scalar.dma_start(out=bias_sb, in_=biasv[:, :])
            silu_sb = consts.tile([JPC, N], F16, name="silu_sb")
            nc.scalar.dma_start(out=silu_sb, in_=siluT[:, :])
            rhs_sb = consts.tile([128, NOCT * 512], F16, name="rhs_sb")
            nc.sync.dma_start(out=rhs_sb, in_=rhsp[:, :])

            ss_tiles = [None] * NOCT
            chain_t = [None] * NOCT
            cnt = 0

            def emit_chain_piece(o, lo, hi):
                # relu (scalar) -> square (gpsimd) -> mul (vector, fp16 out)
                # over cols [lo, hi), then silu rows 30/31 scatter (sync).
                if chain_t[o] is None:
                    t1 = chain.tile([128, N], F32, tag="t1", name=f"t1_{o}")
                    t2 = chain.tile([128, N], F32, tag="t2", name=f"t2_{o}")
                    chain_t[o] = (t1, t2)
                    ss_tiles[o] = sspool.tile([128, N], F16, tag=f"ss{o}",
                                              name=f"ss{o}")
                t1, t2 = chain_t[o]
                ss = ss_tiles[o]
                nc.scalar.activation(
                    t1[:, lo:hi], xr_tiles[o][:, lo:hi],
                    mybir.ActivationFunctionType.Relu,
                    bias=bias_sb[:, 0:1], scale=scale_val,
                )
                nc.gpsimd.tensor_mul(t2[:, lo:hi], t1[:, lo:hi], t1[:, lo:hi])
                nc.vector.tensor_mul(ss[:, lo:hi], t1[:, lo:hi], t2[:, lo:hi])
                for r in range(4):
                    nc.sync.dma_start(
                        out=ss[32 * r + 30 : 32 * r + 32, lo:hi],
                        in_=silu_sb[8 * o + 2 * r : 8 * o + 2 * r + 2, lo:hi],
                    )

            def emit_main_chunk(o, c, last=False):
                nonlocal cnt
                ss = ss_tiles[o]
                st = stage_pool.tile([128, 4096], F16, tag="st",
                                     name=f"st{o}_{c}")
                for t in range(2):       # row residue: n = 256c + 2p + t
                    ps = psum_pool.tile([128, 2048], F32, tag="ps",
                                        name=f"ps{o}_{c}_{t}")
                    for r in range(4):
                        nc.tensor.matmul(
                            ps[:, 512 * r : 512 * (r + 1)],
                            lhsT=ss[32 * r : 32 * r + 32,
                                    256 * c + t : 256 * (c + 1) : 2],
                            rhs=rhs_sb[32 * r : 32 * r + 32,
                                       512 * o : 512 * (o + 1)],
                            start=True,
                            stop=True,
                            tile_position=(32 * r, 0),
                        )
                    dst = st[:, 2048 * t : 2048 * (t + 1)]
                    if cnt % EVAC_PERIOD in EVAC_ACT:
                        nc.scalar.copy(dst, ps)
                    else:
                        nc.vector.tensor_copy(dst, ps)
                    cnt += 1
                    if not last:
                        deng = nc.sync if t == 0 else nc.gpsimd
                        deng.dma_start(out=out[o, c, t], in_=dst)
                    else:
                        # Drain: quarter stores across both queues.
                        h = 1024
                        nc.sync.dma_start(out=out[o, c, t, :, 0:h],
                                          in_=dst[:, 0:h])
                        nc.gpsimd.dma_start(out=out[o, c, t, :, h:2048],
                                            in_=dst[:, h:2048])

            # Octet 0 fill: quarter chain pieces, each feeding 2 chunks.
            emit_chain_piece(0, 0, 512)
            # Remaining x replicas load behind the octet-0 critical path
            # (before any store-gen blocks the gpsimd queue).
            for o in range(1, NOCT):
                nc.gpsimd.dma_start(out=xr_tiles[o], in_=xrep[o])
            emit_main_chunk(0, 0)
            emit_main_chunk(0, 1)
            for piece in range(1, 4):
                emit_chain_piece(0, 512 * piece, 512 * (piece + 1))
                emit_main_chunk(0, 2 * piece)
                emit_main_chunk(0, 2 * piece + 1)

            # Wavefront: octet o's chunks carry octet o+1's chain halves.
            for o in range(NOCT):
                for c in range(NCH):
                    if o > 0:
                        emit_main_chunk(o, c, last=(o == NOCT - 1 and
                                                    c == NCH - 1))
                    if o + 1 < NOCT and c in (2, 5):
                        lo = 0 if c == 2 else N // 2
                        emit_chain_piece(o + 1, lo, lo + N // 2)

    nc.compile()
    return nc


def _host_prep(x, C, W, grid):
    """Build per-core input maps."""
    t0 = np.float64(grid[0, 0])
    h = np.float64(grid[0, 1] - grid[0, 0])
    cbrt = np.float64(WSCALE) ** (1.0 / 3.0)
    w5 = np.array([1.0, -4.0, 6.0, -4.0, 1.0], np.float64) / 6.0

    # Fold the banded combine into the weights (f64):
    #   Cw'[p, col] = sum_f M[p, f] * (C*W)[f, col],  M[f+r, f] = w5[r].
    M = np.zeros((15, 11), np.float64)
    for f in range(11):
        for r in range(5):
            M[f + r, f] = w5[r]
    CW = C.astype(np.float64) * W.astype(np.float64)        # (11, 65536)
    Cwp32 = (M @ CW * WSCALE).astype(np.float16)            # (15, 65536)
    W32 = (W.astype(np.float64) * WSCALE).astype(np.float16)

    xd = x.astype(np.float64)
    silu_p = (xd / (1.0 + np.exp(-xd)) / WSCALE).astype(np.float16)  # silu/32

    # S-tile partition layout within a 32-row group:
    #   s in [0,15)  -> S_i of j_a (i = s)
    #   s in [15,30) -> S_i of j_b (i = s - 15)
    #   s = 30/31    -> silu'(j_a)/silu'(j_b) (scatter; relu bias -64 ->
    #                   the chain writes exact zeros there first)
    s_idx = np.arange(128) % 32
    feat_i = np.where(s_idx < 15, s_idx, np.where(s_idx < 30, s_idx - 15, 0))
    which_b = np.where(s_idx < 15, 0, np.where(s_idx < 30, 1, s_idx - 30))
    biasv = np.where(
        s_idx < 30, (-t0 / h - feat_i) / cbrt, -64.0
    ).astype(np.float32).reshape(128, 1)
    scale_val = float(np.float32(1.0 / h / cbrt))

    x16 = x.astype(np.float16)
    in_maps = []
    for s in range(NCORES):
        jb = JPC * s
        xt = np.ascontiguousarray(x16[:, jb : jb + JPC].T)    # (32, N) fp16
        xrep = np.empty((NOCT, 128, N), np.float16)
        rgrp = np.arange(128) // 32
        for o in range(NOCT):
            jloc = 8 * o + 2 * rgrp + which_b
            xrep[o] = xt[jloc]
        silu_t = np.ascontiguousarray(silu_p[:, jb : jb + JPC].T)  # (32, N)

        # rhs row layout per group: [15 Cw'a, 15 Cw'b, W a, W b] (x32)
        rhsp = np.zeros((128, NOCT * 512), np.float16)
        for o in range(NOCT):
            for rr in range(4):
                ja = (jb + 8 * o + 2 * rr) * N_OUT
                jbc = (jb + 8 * o + 2 * rr + 1) * N_OUT
                base = 32 * rr
                rhsp[base : base + 15, 512 * o : 512 * o + 256] = \
                    Cwp32[:, ja : ja + 256]
                rhsp[base + 15 : base + 30, 512 * o + 256 : 512 * o + 512] = \
                    Cwp32[:, jbc : jbc + 256]
                rhsp[base + 30, 512 * o : 512 * o + 256] = W32[0, ja : ja + 256]
                rhsp[base + 31, 512 * o + 256 : 512 * o + 512] = \
                    W32[0, jbc : jbc + 256]
        in_maps.append({
            "xrep": np.ascontiguousarray(xrep),
            "biasv": biasv,
            "rhsp": np.ascontiguousarray(rhsp),
            "siluT": silu_t,
        })
    return in_maps, scale_val


def _assemble(out_core):
    """[NOCT, NCH, 2, 128, 2048] fp16 -> [N, 8192] (n = 256c + 2p + t)."""
    a = out_core.reshape(NOCT, NCH, 2, 128, 2048)
    return a.transpose(1, 3, 2, 0, 4).reshape(N, JPC * N_OUT)


def kernel(x, C, W, grid):
    in_maps, scale_val = _host_prep(
        np.asarray(x, np.float32), np.asarray(C, np.float32),
        np.asarray(W, np.float32), np.asarray(grid, np.float32),
    )
    nc = _build_bass(scale_val)
    res = run_bass_kernel_spmd(nc, in_maps, core_ids=list(range(NCORES)))
    return np.ascontiguousarray(np.concatenate(
        [_assemble(r["out"]).astype(np.float32) for r in res.results], axis=1))


if __name__ == "__main__":
    rng = np.random.default_rng(0)
    x = rng.standard_normal((N, N_IN), dtype=np.float32)
    C = rng.standard_normal((11, N_IN * N_OUT), dtype=np.float32) * 0.005
    W = rng.standard_normal((1, N_IN * N_OUT), dtype=np.float32) * 0.005
    knots = -5.25 + 0.75 * np.arange(15, dtype=np.float32)
    grid = np.tile(knots, (N_IN, 1))
    out = kernel(x, C, W, grid)
    print("kernel out:", out.shape, out.dtype, float(np.abs(out).mean()))
